# revision 64
# baseline (speedup 1.0000x reference)
"""Trainium2 Bass kernel for nn_MultiHeadSelfAttention_88725434400988.

Self-contained: accepts FULL inputs, shards batch B=256 over 8 NeuronCores
(32 per core), runs one SPMD Bass program, gathers the FULL output.

Per-core algorithm (B_CORE=32, S=8, F=32, E=64, A=64, NH=2), fp16 operands
with fp32 PSUM accumulation.  ~186us HW exec (vs 226us baseline).  The
schedule keeps the PE HAM clock gate at K=8/8 (2.4 GHz) for the whole
kernel: any ~3.4us activity window that is mostly idle re-throttles the PE
to 1.2 GHz, so the PE stream never has a gap longer than ~1.5us.

  - Q/K projection: lhsT = 128-col tiles of W (FWL fp16), rhs = Hs^T;
    2-tile psum groups stream at the PE roofline (109ns per N=256 matmul).
    hst leads the sync HWDGE ring and wt0 the scalar ring (the two 1MB
    critical-path transfers run in parallel); wt1-7 carry tile_wait_until
    clock waits so the scheduler cannot hoist their transfers into the
    head; 8-deep pool prefetch (4MB cushion) rides out the ~4% deficit
    between sustained SDMA rate and PE weight consumption.
  - Projection psum evacuation routes the four (jh, nh) quadrants so the
    attention tiles get partition = nh*64+a: same-half quadrants copy
    straight into qt/kt (VectorE lower half, ScalarE Copy upper half),
    cross-half quadrants stage and then partition-shift via contiguous
    b-chunked SBUF->SBUF DMAs (gpsimd ring for Wq, sync for Wk so z(b)
    only waits ~1us for its own chunk at the transition).
  - v projection rides INSIDE the projection stream (one full-mode K=128
    matmul per weight-tile slot against a block-diagonal [128,256] Wv
    resolves two batches with no PE mode switch); dead-matmul fillers
    cover the proj->attention transition and the first exp's latency.
  - Attention is transpose-free (Z^T layout) in pipelined 2-batch groups,
    one pair AHEAD of the denominator/AV/normalize stage, so ScalarE (one
    fused N=1024 exp ACTIVATE per batch, ~1.15us -- the bottleneck) never
    starves and the PE fills the exp latency with the previous pair's
    work.  qt/kt partition = nh*64+a lets the two heads' z-matmuls run
    concurrently as 64x128 row tiles into different psum banks.
  - Denominators land replicated on the right partition halves straight
    from the PE (lhsT = ones(128,64), col-packed per nh); one
    reciprocal_approx_fast per batch; normalize multiply on VectorE.
  - Residual: block-diagonal Wres col-tiled over e-halves shares the
    den/AV 128x64 mode (no mode-switch drain) and is deferred one pair so
    the PE never waits on the DVE normalize; relu+bias on VectorE
    (tensor_scalar add+max) keeps ScalarE exp-only; output DMAs on sync.
  - A dummy 1-element Exp right after wt0's DMA issue pulls the ACT table
    load (~2.7us of ScalarE) into the DMA-wait head.
"""
import numpy as np

B, S, F, E, A, NH = 256, 8, 32, 64, 64, 2
NCORES = 8
BC = B // NCORES            # 32 batches per core
ROWS = BC * S               # 256 projection rows
CD = F * E                  # 2048 contraction dim
ND = A * F * NH             # 4096 projection cols
KTILES = CD // 128          # 16
TTILES = ND // 128          # 32 column tiles per weight
NB = BC * NH                # 64 attention batches per core
WCHUNK = 2                  # weight tiles per DMA / psum group
NGRP = TTILES // WCHUNK     # 16 tile groups per weight
NSLOT = 2 * NGRP            # 32 total projection slots
VLAG = 4                    # v-projection trails the slot index by this

_NC_CACHE = None


def build_bass():
    import concourse.bacc as bacc
    import concourse.tile as tile
    from concourse import mybir

    f16 = mybir.dt.float16
    bf16 = mybir.dt.bfloat16
    f32 = mybir.dt.float32
    Exp = mybir.ActivationFunctionType.Exp
    Copy = mybir.ActivationFunctionType.Copy
    Add = mybir.AluOpType.add
    Max = mybir.AluOpType.max

    nc = bacc.Bacc("TRN2", target_bir_lowering=False, debug=False)

    # host-prepped layouts (see make_in_maps)
    hst_d = nc.dram_tensor("hst", [128, KTILES, ROWS], f16, kind="ExternalInput")
    hsv_d = nc.dram_tensor("hsv", [128, NB // 2, 128], f16, kind="ExternalInput")
    wq_d = nc.dram_tensor("wq", [128, TTILES, KTILES * 128], f16,
                          kind="ExternalInput")
    wk_d = nc.dram_tensor("wk", [128, TTILES, KTILES * 128], f16,
                          kind="ExternalInput")
    wv_d = nc.dram_tensor("wv", [128, 256], f16, kind="ExternalInput")
    wres_d = nc.dram_tensor("wres", [128, 2, 128], f16, kind="ExternalInput")
    bias_d = nc.dram_tensor("bias", [128, 1], f32, kind="ExternalInput")
    out_d = nc.dram_tensor("out", [128, BC * S * F // 2], f32,
                           kind="ExternalOutput")

    with tile.TileContext(nc) as tc:
        from contextlib import ExitStack
        with ExitStack() as ctx:
            singles = ctx.enter_context(tc.tile_pool(name="singles", bufs=1))

            # ---- constants / persistent tiles ----
            ones_bf = singles.tile([128, A], bf16)
            nc.vector.memset(ones_bf, 1.0)
            dummy = singles.tile([1, 1], f32)

            # hst owns the sync HWDGE ring early; wt0 goes on the scalar
            # ring so the two 1MB critical transfers run in parallel.
            hsT = singles.tile([128, KTILES, ROWS], f16)
            nc.sync.dma_start(hsT[:, 0:8, :], hst_d[:, 0:8, :])
            nc.sync.dma_start(hsT[:, 8:16, :], hst_d[:, 8:16, :])
            hsv = singles.tile([128, NB // 2, 128], f16)

            wv_sb = singles.tile([128, 256], f16)
            wres_sb = singles.tile([128, 2, 128], f16)
            bias_sb = singles.tile([128, 1], f32)

            # (nh*64+a, jh, b, f*4+sp): nh on partition halves lets the two
            # heads' z-matmuls run CONCURRENTLY as 64x128 row tiles.
            qt = singles.tile([128, 2, BC, 128], f16)
            kt_ = singles.tile([128, 2, BC, 128], f16)
            v_all = singles.tile([128, NB, 2, A], bf16)
            ut = singles.tile([128, BC, 2, 128], f16)  # (nh,a) x (b, jh, f*4+sp)

            # ---- Q/K projection with embedded v projection ----
            with tc.tile_pool(name="wtile", bufs=8) as w_pool, \
                 tc.tile_pool(name="stage", bufs=2) as st_pool, \
                 tc.tile_pool(name="pp", bufs=3, space="PSUM") as pp_pool, \
                 tc.tile_pool(name="vps", bufs=2, space="PSUM") as vps_pool:

                wts = {}

                def issue_wt(slot, eng=None):
                    w_d = wq_d if slot < NGRP else wk_d
                    tg = slot % NGRP
                    wt = w_pool.tile([128, WCHUNK, KTILES, 128], f16,
                                     name="wt", tag="wt")
                    (eng or nc.sync).dma_start(
                        wt[:, :, :, :],
                        w_d[:, tg * WCHUNK:(tg + 1) * WCHUNK, :]
                        .rearrange("p t (kt c) -> p t kt c", c=128))
                    wts[slot] = wt

                def emit_v(q):
                    # one full-mode matmul resolves v for batches 2q, 2q+1
                    # (block-diagonal wv: psum cols 0:128 -> bn=2q, 128:256
                    # -> bn=2q+1)
                    vp = vps_pool.tile([128, 256], f32, name="vp", tag="vp")
                    nc.tensor.matmul(vp[:, :], lhsT=hsv[:, q, :],
                                     rhs=wv_sb[:, :], start=True, stop=True)
                    nc.vector.tensor_copy(
                        v_all[:, 2 * q:2 * q + 2, :, :]
                        .rearrange("p q k a -> p (q k a)"),
                        vp[:, :])

                # wt0 rides the scalar ring (parallel with hst on sync);
                # wt1-7 carry clock waits so the scheduler cannot hoist
                # their transfers into the head where they would steal SDMA
                # bandwidth from the two critical-path megabytes.  8-deep
                # pool prefetch (4MB cushion) absorbs the ~4% deficit
                # between sustained SDMA rate and PE weight consumption.
                issue_wt(0, eng=nc.scalar)
                # dummy exp pulls the ACT table load (~2.7us, blocks the
                # scalar engine) into the DMA-wait head AFTER wt0's issue
                nc.scalar.activation(dummy[:, :], ones_bf[0:1, 0:1], Exp)
                nc.scalar.dma_start(wv_sb[:, :], wv_d[:])
                nc.scalar.dma_start(wres_sb[:, :, :], wres_d[:])
                nc.scalar.dma_start(bias_sb[:, :], bias_d[:])
                for s in range(1, 8):
                    with tc.tile_wait_until(0.007 + 0.002 * s):
                        issue_wt(s)

                stage = None
                for slot in range(NSLOT):
                    widx, tg = divmod(slot, NGRP)
                    dest = qt if widx == 0 else kt_
                    if tg == 0:
                        stage = st_pool.tile([128, BC, 128], f16,
                                             name="stage", tag="stage")
                    if slot + 8 < NSLOT:
                        issue_wt(slot + 8)
                    wt = wts.pop(slot)
                    pp = pp_pool.tile([128, WCHUNK, ROWS], f32,
                                      name="pp", tag="pp")
                    for ti in range(WCHUNK):
                        for kt in range(KTILES):
                            nc.tensor.matmul(
                                pp[:, ti, :],
                                lhsT=wt[:, ti, kt, :],
                                rhs=hsT[:, kt, :],
                                start=(kt == 0),
                                stop=(kt == KTILES - 1))
                    if VLAG <= slot < VLAG + NB // 2:
                        emit_v(slot - VLAG)
                    # psum partition = jh*64+a, free rows (b, nh, sp).  Four
                    # evac copies route each (jh, nh) quadrant: same-half
                    # quadrants go straight into qt/kt (partition = nh*64+a),
                    # cross-half quadrants go to stage for the partition-
                    # shift DMA.  VectorE takes the lower psum half, ScalarE
                    # the upper.
                    t0 = tg * WCHUNK
                    src = pp.rearrange(
                        "p ti (b n sp) -> p b n ti sp", n=NH, sp=4)
                    dq = dest.rearrange("p jh b (f sp) -> p jh b f sp", sp=4)
                    sg = stage.rearrange("p b (f sp) -> p b f sp", sp=4)
                    nc.vector.tensor_copy(
                        dq[0:64, 0, :, t0:t0 + WCHUNK, :],
                        src[0:64, :, 0, :, :])
                    nc.vector.tensor_copy(
                        sg[0:64, :, t0:t0 + WCHUNK, :],
                        src[0:64, :, 1, :, :])
                    nc.scalar.activation(
                        dq[64:128, 1, :, t0:t0 + WCHUNK, :],
                        src[64:128, :, 1, :, :], Copy)
                    nc.scalar.activation(
                        sg[64:128, :, t0:t0 + WCHUNK, :],
                        src[64:128, :, 0, :, :], Copy)
                    # hsv on the scalar ring with clock waits so its
                    # transfers cannot overlap wt0's critical tail
                    if slot in (0, 1):
                        for hc in range(2 * slot, 2 * slot + 2):
                            with tc.tile_wait_until(0.008 + 0.0005 * hc):
                                nc.scalar.dma_start(
                                    hsv[:, hc * 8:(hc + 1) * 8, :],
                                    hsv_d[:, hc * 8:(hc + 1) * 8, :])
                    # partition shifts (engines cannot cross partitions; DMA
                    # can), contiguous both sides, in b-chunks so z(b) only
                    # waits for its own chunk.  Wq's go on the gpsimd ring
                    # (overlap the Wk stream); Wk's on sync for minimum
                    # latency at the proj->attention transition.
                    if tg == NGRP - 1:
                        dma_eng = nc.gpsimd if widx == 0 else nc.sync
                        for ci in range(2):
                            bs = slice(ci * 16, (ci + 1) * 16)
                            nc_e = dma_eng
                            nc_e.dma_start(dest[64:128, 0, bs, :],
                                           stage[0:64, bs, :])
                            nc_e.dma_start(dest[0:64, 1, bs, :],
                                           stage[64:128, bs, :])

                # v tail keeps the PE busy while the last kt shift lands
                for q in range(NSLOT - VLAG, NB // 2):
                    emit_v(q)
                # dead-matmul filler: the HAM gate re-throttles if a ~3.4us
                # activity window is mostly idle.  The kt shift + first exp
                # leave the PE thinly occupied for ~3us right here; ~2us of
                # dependency-free matmuls keep the window busy at zero span
                # cost (the real work isn't ready yet anyway).
                wm = vps_pool.tile([128, 256], f32, name="vp", tag="vp")
                for wi in range(13):
                    nc.tensor.matmul(wm[:, :], lhsT=hsT[:, 0, 0:128],
                                     rhs=hsT[:, 0, :],
                                     start=(wi == 0), stop=(wi == 12))

            # ---- attention (transpose-free, Z^T layout, pipelined pairs) --
            # All four PSUM uses share the zt pool: after the fused exp
            # reads a zt tile its two banks are dead, so the denominator
            # (bank0 lower half), AV (bank0 upper half) and residual
            # (bank1) are aliased into it.  has_written bits are
            # per-element, so same-bank accumulation groups are safe (the
            # projection psum groups already rely on this).  Freed banks
            # let the zt pool go 4-deep: z(b+2)/z(b+3) have no WAR at all,
            # so the pipeline self-covers the exp latency with no fillers.
            with tc.tile_pool(name="zps", bufs=4, space="PSUM") as z_pool, \
                 tc.tile_pool(name="expz", bufs=4) as e_pool, \
                 tc.tile_pool(name="reps", bufs=2) as rp_pool, \
                 tc.tile_pool(name="fo", bufs=2) as f_pool:

                ezs = {}
                zts = {}

                def emit_front(b):
                    # z matmuls + one fused exp over all (nh, h).  The two
                    # heads occupy disjoint 64-row strips of the PE (qt/kt
                    # partition = nh*64+a) and write different psum banks,
                    # so each (nh0, nh1) pair runs concurrently.
                    zt = z_pool.tile([128, 2, 2, 256], f32, name="zt",
                                     tag="zt")
                    zts[b] = zt
                    for h in range(2):
                        for nh in range(NH):
                            nc.tensor.matmul(
                                zt[:, nh, h, :],
                                lhsT=kt_[nh * 64:(nh + 1) * 64, h, b, :],
                                rhs=qt[nh * 64:(nh + 1) * 64, :, b, :],
                                start=True, stop=True)
                    ez = e_pool.tile([128, 2, 2, 256], bf16, name="ez",
                                     tag="ez")
                    ezs[b] = ez
                    nc.scalar.activation(
                        ez.rearrange("p n h t -> p (n h t)"),
                        zt.rearrange("p n h t -> p (n h t)"), Exp)

                def emit_back(b):
                    ez = ezs.pop(b)
                    zt = zts.pop(b)
                    # denominators replicated onto the right partition
                    # halves directly by the PE, into zt bank0 lower half
                    for h in range(2):
                        for nh in range(NH):
                            nc.tensor.matmul(
                                zt[nh * 64:(nh + 1) * 64, 0, 0, :],
                                lhsT=ones_bf[:, :],
                                rhs=ez[:, nh, h, :],
                                start=(h == 0), stop=(h == 1),
                                tile_position=(0, nh * 64))
                    rep = rp_pool.tile([128, 256], f32, name="rep", tag="rep")
                    nc.vector.reciprocal_approx_fast(rep[:, :],
                                                     zt[:, 0, 0, :])
                    # AV into zt bank0 upper half
                    for kk in range(2):
                        for nh in range(NH):
                            bn = b * NH + nh
                            nc.tensor.matmul(
                                zt[nh * 64:(nh + 1) * 64, 0, 1, :],
                                lhsT=v_all[:, bn, kk, :],
                                rhs=ez[:, nh, kk, :],
                                start=(kk == 0), stop=(kk == 1),
                                tile_position=(0, nh * 64))
                    nc.vector.tensor_mul(
                        ut[:, b, :, :].rearrange("p a c -> p (a c)"),
                        zt[:, 0, 1, :], rep[:, :])
                    # residual per completed 4-batch group into zt bank1;
                    # col-tiled (e-halves concurrent) so it shares the
                    # den/AV 128x64 mode; relu+bias on VectorE so ScalarE
                    # only runs exp.
                    if b % 4 == 3:
                        bg = b // 4
                        rp = zt[:, 1, :, :].rearrange("p a c -> p (a c)")
                        for jh in range(2):
                            for eh in range(2):
                                nc.tensor.matmul(
                                    rp[eh * 64:(eh + 1) * 64, :],
                                    lhsT=wres_sb[:, jh, eh * 64:(eh + 1) * 64],
                                    rhs=ut[:, bg * 4:(bg + 1) * 4, jh, :],
                                    start=(jh == 0), stop=(jh == 1),
                                    tile_position=(0, eh * 64))
                        fo = f_pool.tile([128, 512], f32, name="fo", tag="fo")
                        nc.vector.tensor_scalar(
                            fo[:, :], rp[:, :], bias_sb[:, :], 0.0, Add, Max)
                        nc.sync.dma_start(
                            out_d[:, bg * 512:(bg + 1) * 512], fo[:, :])

                for p in range(BC // 2 + 1):
                    if p < BC // 2:
                        emit_front(2 * p)
                        emit_front(2 * p + 1)
                    if p >= 1:
                        emit_back(2 * (p - 1))
                        emit_back(2 * (p - 1) + 1)
    nc.compile()
    return nc


def _get_nc():
    global _NC_CACHE
    if _NC_CACHE is None:
        _NC_CACHE = build_bass()
    return _NC_CACHE


def _prep_weight(W):
    # (CD, ND) -> (128, TTILES, KTILES*128): [p, t, kt*128+j] = W[kt*128+p, t*128+j]
    return np.ascontiguousarray(
        W.astype(np.float16).reshape(KTILES, 128, TTILES, 128)
        .transpose(1, 2, 0, 3).reshape(128, TTILES, KTILES * 128))


def make_in_maps(Hs, Wq, Wk, Wv, Wres_w, Wres_b):
    wq16 = _prep_weight(Wq)
    wk16 = _prep_weight(Wk)
    # block-diagonal Wv: [128, 256], partitions (pi, e), cols (pi, 2A)
    wv16 = np.zeros((128, 256), np.float16)
    wv16[0:64, 0:128] = Wv.astype(np.float16)
    wv16[64:128, 128:256] = Wv.astype(np.float16)
    # block-diagonal Wres: [p=(nh,a), jh, col=(nh,e)]
    wres16 = np.zeros((128, 2, 128), np.float16)
    for jh in range(2):
        for nh in range(2):
            wres16[nh * 64:(nh + 1) * 64, jh, nh * 64:(nh + 1) * 64] = \
                Wres_w[jh * 64:(jh + 1) * 64, :].astype(np.float16)
    bias = np.tile(Wres_b.astype(np.float32), 2).reshape(128, 1)
    hs16 = Hs.astype(np.float16)
    maps = []
    for c in range(NCORES):
        sh = hs16[c * BC:(c + 1) * BC]                      # (BC, S, CD)
        hs2d = sh.reshape(ROWS, CD)
        hst = np.ascontiguousarray(
            hs2d.reshape(ROWS, KTILES, 128).transpose(2, 1, 0))
        # v rows in sigma' order (f*4+sp):
        # hsv[pi*64+e, q, f*4+sp] = Hs[b, nh*4+sp, f, e]; bn = 2q+pi = b*NH+nh
        arr = sh.reshape(NB, 4, F, E).transpose(0, 2, 1, 3).reshape(NB, 128, E)
        hsv = np.ascontiguousarray(
            arr.reshape(NB // 2, 2, 128, E).transpose(1, 3, 0, 2)
            .reshape(128, NB // 2, 128))
        maps.append({
            "hst": hst, "hsv": hsv,
            "wq": wq16, "wk": wk16, "wv": wv16, "wres": wres16, "bias": bias,
        })
    return maps


def _unpack_out(o):
    # o: (128, 4096) = (nh*64+e, bg, b4, f, sp) -> (BC, S, F*E)
    o = o.reshape(NH, E, BC // 4, 4, F, 4)
    return np.ascontiguousarray(
        o.transpose(2, 3, 0, 5, 4, 1)).reshape(BC, S, F * E)


def kernel(Hs, Wq, Wk, Wv, Wres_w, Wres_b):
    from concourse.bass_utils import run_bass_kernel_spmd
    nc = _get_nc()
    in_maps = make_in_maps(Hs, Wq, Wk, Wv, Wres_w, Wres_b)
    res = run_bass_kernel_spmd(nc, in_maps, list(range(NCORES)))
    out = np.concatenate(
        [_unpack_out(np.asarray(res.results[c]["out"]))
         for c in range(NCORES)], axis=0)
    return out.astype(np.float32)


if __name__ == "__main__":
    nc = build_bass()
    print("built OK; instructions:",
          sum(len(bb.instructions) for fn in nc.m.functions
              for bb in fn.blocks))


# revision 65
# speedup vs baseline: 1.0371x; 1.0371x over previous
"""Trainium2 Bass kernel for nn_MultiHeadSelfAttention_88725434400988.

Self-contained: accepts FULL inputs, shards batch B=256 over 8 NeuronCores
(32 per core), runs one SPMD Bass program, gathers the FULL output.

Per-core algorithm (B_CORE=32, S=8, F=32, E=64, A=64, NH=2), fp16 operands
with fp32 PSUM accumulation.  ~186us HW exec (vs 226us baseline).  The
schedule keeps the PE HAM clock gate at K=8/8 (2.4 GHz) for the whole
kernel: any ~3.4us activity window that is mostly idle re-throttles the PE
to 1.2 GHz, so the PE stream never has a gap longer than ~1.5us.

  - Q/K projection: lhsT = 128-col tiles of W (FWL fp16), rhs = Hs^T;
    2-tile psum groups stream at the PE roofline (109ns per N=256 matmul).
    hst leads the sync HWDGE ring and wt0 the scalar ring (the two 1MB
    critical-path transfers run in parallel); wt1-7 carry tile_wait_until
    clock waits so the scheduler cannot hoist their transfers into the
    head; 8-deep pool prefetch (4MB cushion) rides out the ~4% deficit
    between sustained SDMA rate and PE weight consumption.
  - Projection psum evacuation routes the four (jh, nh) quadrants so the
    attention tiles get partition = nh*64+a: same-half quadrants copy
    straight into qt/kt (VectorE lower half, ScalarE Copy upper half),
    cross-half quadrants stage and then partition-shift via contiguous
    b-chunked SBUF->SBUF DMAs (gpsimd ring for Wq, sync for Wk so z(b)
    only waits ~1us for its own chunk at the transition).
  - v projection rides INSIDE the projection stream (one full-mode K=128
    matmul per weight-tile slot against a block-diagonal [128,256] Wv
    resolves two batches with no PE mode switch); dead-matmul fillers
    cover the proj->attention transition and the first exp's latency.
  - Attention is transpose-free (Z^T layout) in pipelined 2-batch groups,
    one pair AHEAD of the denominator/AV/normalize stage, so ScalarE (one
    fused N=1024 exp ACTIVATE per batch, ~1.15us -- the bottleneck) never
    starves and the PE fills the exp latency with the previous pair's
    work.  qt/kt partition = nh*64+a lets the two heads' z-matmuls run
    concurrently as 64x128 row tiles into different psum banks.
  - Denominators land replicated on the right partition halves straight
    from the PE (lhsT = ones(128,64), col-packed per nh); one
    reciprocal_approx_fast per batch; normalize multiply on VectorE.
  - Residual: block-diagonal Wres col-tiled over e-halves shares the
    den/AV 128x64 mode (no mode-switch drain) and is deferred one pair so
    the PE never waits on the DVE normalize; relu+bias on VectorE
    (tensor_scalar add+max) keeps ScalarE exp-only; output DMAs on sync.
  - A dummy 1-element Exp right after wt0's DMA issue pulls the ACT table
    load (~2.7us of ScalarE) into the DMA-wait head.
"""
import numpy as np

B, S, F, E, A, NH = 256, 8, 32, 64, 64, 2
NCORES = 8
BC = B // NCORES            # 32 batches per core
ROWS = BC * S               # 256 projection rows
CD = F * E                  # 2048 contraction dim
ND = A * F * NH             # 4096 projection cols
KTILES = CD // 128          # 16
TTILES = ND // 128          # 32 column tiles per weight
NB = BC * NH                # 64 attention batches per core
WCHUNK = 2                  # weight tiles per DMA / psum group
NGRP = TTILES // WCHUNK     # 16 tile groups per weight
NSLOT = 2 * NGRP            # 32 total projection slots
VLAG = 4                    # v-projection trails the slot index by this

_NC_CACHE = None


def build_bass():
    import concourse.bacc as bacc
    import concourse.tile as tile
    from concourse import mybir

    f16 = mybir.dt.float16
    bf16 = mybir.dt.bfloat16
    f32 = mybir.dt.float32
    Exp = mybir.ActivationFunctionType.Exp
    Copy = mybir.ActivationFunctionType.Copy
    Add = mybir.AluOpType.add
    Max = mybir.AluOpType.max

    nc = bacc.Bacc("TRN2", target_bir_lowering=False, debug=False)

    # host-prepped layouts (see make_in_maps)
    hst_d = nc.dram_tensor("hst", [128, KTILES, ROWS], f16, kind="ExternalInput")
    hsv_d = nc.dram_tensor("hsv", [128, NB // 2, 128], f16, kind="ExternalInput")
    wq_d = nc.dram_tensor("wq", [128, TTILES, KTILES * 128], f16,
                          kind="ExternalInput")
    wk_d = nc.dram_tensor("wk", [128, TTILES, KTILES * 128], f16,
                          kind="ExternalInput")
    wv_d = nc.dram_tensor("wv", [128, 256], f16, kind="ExternalInput")
    wres_d = nc.dram_tensor("wres", [128, 2, 128], f16, kind="ExternalInput")
    bias_d = nc.dram_tensor("bias", [128, 1], f32, kind="ExternalInput")
    out_d = nc.dram_tensor("out", [128, BC * S * F // 2], f32,
                           kind="ExternalOutput")

    with tile.TileContext(nc) as tc:
        from contextlib import ExitStack
        with ExitStack() as ctx:
            singles = ctx.enter_context(tc.tile_pool(name="singles", bufs=1))

            # ---- constants / persistent tiles ----
            ones_bf = singles.tile([128, A], bf16)
            nc.vector.memset(ones_bf, 1.0)
            dummy = singles.tile([1, 1], f32)

            # hst owns the sync HWDGE ring early; wt0 goes on the scalar
            # ring so the two 1MB critical transfers run in parallel.
            hsT = singles.tile([128, KTILES, ROWS], f16)
            nc.sync.dma_start(hsT[:, 0:8, :], hst_d[:, 0:8, :])
            nc.sync.dma_start(hsT[:, 8:16, :], hst_d[:, 8:16, :])
            hsv = singles.tile([128, NB // 2, 128], f16)

            wv_sb = singles.tile([128, 256], f16)
            wres_sb = singles.tile([128, 2, 128], f16)
            bias_sb = singles.tile([128, 1], f32)

            # (nh*64+a, jh, b, f*4+sp): nh on partition halves lets the two
            # heads' z-matmuls run CONCURRENTLY as 64x128 row tiles.
            qt = singles.tile([128, 2, BC, 128], f16)
            kt_ = singles.tile([128, 2, BC, 128], f16)
            v_all = singles.tile([128, NB, 2, A], bf16)
            ut = singles.tile([128, BC, 2, 128], f16)  # (nh,a) x (b, jh, f*4+sp)

            # ---- Q/K projection with embedded v projection ----
            with tc.tile_pool(name="wtile", bufs=8) as w_pool, \
                 tc.tile_pool(name="stage", bufs=2) as st_pool, \
                 tc.tile_pool(name="pp", bufs=3, space="PSUM") as pp_pool, \
                 tc.tile_pool(name="vps", bufs=2, space="PSUM") as vps_pool:

                wts = {}

                def issue_wt(slot, eng=None):
                    w_d = wq_d if slot < NGRP else wk_d
                    tg = slot % NGRP
                    wt = w_pool.tile([128, WCHUNK, KTILES, 128], f16,
                                     name="wt", tag="wt")
                    (eng or nc.sync).dma_start(
                        wt[:, :, :, :],
                        w_d[:, tg * WCHUNK:(tg + 1) * WCHUNK, :]
                        .rearrange("p t (kt c) -> p t kt c", c=128))
                    wts[slot] = wt

                def emit_v(q):
                    # one full-mode matmul resolves v for batches 2q, 2q+1
                    # (block-diagonal wv: psum cols 0:128 -> bn=2q, 128:256
                    # -> bn=2q+1)
                    vp = vps_pool.tile([128, 256], f32, name="vp", tag="vp")
                    nc.tensor.matmul(vp[:, :], lhsT=hsv[:, q, :],
                                     rhs=wv_sb[:, :], start=True, stop=True)
                    nc.vector.tensor_copy(
                        v_all[:, 2 * q:2 * q + 2, :, :]
                        .rearrange("p q k a -> p (q k a)"),
                        vp[:, :])

                # wt0 rides the scalar ring (parallel with hst on sync);
                # wt1-7 carry clock waits so the scheduler cannot hoist
                # their transfers into the head where they would steal SDMA
                # bandwidth from the two critical-path megabytes.  8-deep
                # pool prefetch (4MB cushion) absorbs the ~4% deficit
                # between sustained SDMA rate and PE weight consumption.
                issue_wt(0, eng=nc.scalar)
                # dummy exp pulls the ACT table load (~2.7us, blocks the
                # scalar engine) into the DMA-wait head AFTER wt0's issue
                nc.scalar.activation(dummy[:, :], ones_bf[0:1, 0:1], Exp)
                nc.scalar.dma_start(wv_sb[:, :], wv_d[:])
                nc.scalar.dma_start(wres_sb[:, :, :], wres_d[:])
                nc.scalar.dma_start(bias_sb[:, :], bias_d[:])
                for s in range(1, 8):
                    with tc.tile_wait_until(0.007 + 0.002 * s):
                        issue_wt(s)

                stage = None
                for slot in range(NSLOT):
                    widx, tg = divmod(slot, NGRP)
                    dest = qt if widx == 0 else kt_
                    if tg == 0:
                        stage = st_pool.tile([128, BC, 128], f16,
                                             name="stage", tag="stage")
                    if slot + 8 < NSLOT:
                        issue_wt(slot + 8)
                    wt = wts.pop(slot)
                    pp = pp_pool.tile([128, WCHUNK, ROWS], f32,
                                      name="pp", tag="pp")
                    for ti in range(WCHUNK):
                        for kt in range(KTILES):
                            nc.tensor.matmul(
                                pp[:, ti, :],
                                lhsT=wt[:, ti, kt, :],
                                rhs=hsT[:, kt, :],
                                start=(kt == 0),
                                stop=(kt == KTILES - 1))
                    if VLAG <= slot < VLAG + NB // 2:
                        emit_v(slot - VLAG)
                    # psum partition = jh*64+a, free rows (b, nh, sp).  Four
                    # evac copies route each (jh, nh) quadrant: same-half
                    # quadrants go straight into qt/kt (partition = nh*64+a),
                    # cross-half quadrants go to stage for the partition-
                    # shift DMA.  VectorE takes the lower psum half, ScalarE
                    # the upper.
                    t0 = tg * WCHUNK
                    src = pp.rearrange(
                        "p ti (b n sp) -> p b n ti sp", n=NH, sp=4)
                    dq = dest.rearrange("p jh b (f sp) -> p jh b f sp", sp=4)
                    sg = stage.rearrange("p b (f sp) -> p b f sp", sp=4)
                    nc.vector.tensor_copy(
                        dq[0:64, 0, :, t0:t0 + WCHUNK, :],
                        src[0:64, :, 0, :, :])
                    nc.vector.tensor_copy(
                        sg[0:64, :, t0:t0 + WCHUNK, :],
                        src[0:64, :, 1, :, :])
                    nc.scalar.activation(
                        dq[64:128, 1, :, t0:t0 + WCHUNK, :],
                        src[64:128, :, 1, :, :], Copy)
                    nc.scalar.activation(
                        sg[64:128, :, t0:t0 + WCHUNK, :],
                        src[64:128, :, 0, :, :], Copy)
                    # hsv on the scalar ring with clock waits so its
                    # transfers cannot overlap wt0's critical tail
                    if slot in (0, 1):
                        for hc in range(2 * slot, 2 * slot + 2):
                            with tc.tile_wait_until(0.008 + 0.0005 * hc):
                                nc.scalar.dma_start(
                                    hsv[:, hc * 8:(hc + 1) * 8, :],
                                    hsv_d[:, hc * 8:(hc + 1) * 8, :])
                    # partition shifts (engines cannot cross partitions; DMA
                    # can), contiguous both sides, in b-chunks so z(b) only
                    # waits for its own chunk.  Wq's go on the gpsimd ring
                    # (overlap the Wk stream); Wk's on sync for minimum
                    # latency at the proj->attention transition.
                    if tg == NGRP - 1:
                        dma_eng = nc.gpsimd if widx == 0 else nc.sync
                        for ci in range(2):
                            bs = slice(ci * 16, (ci + 1) * 16)
                            nc_e = dma_eng
                            nc_e.dma_start(dest[64:128, 0, bs, :],
                                           stage[0:64, bs, :])
                            nc_e.dma_start(dest[0:64, 1, bs, :],
                                           stage[64:128, bs, :])

                # v tail keeps the PE busy while the last kt shift lands
                for q in range(NSLOT - VLAG, NB // 2):
                    emit_v(q)
                # dead-matmul filler: the HAM gate re-throttles if a ~3.4us
                # activity window is mostly idle.  The kt shift + first exp
                # leave the PE thinly occupied for ~3us right here; ~2us of
                # dependency-free matmuls keep the window busy at zero span
                # cost (the real work isn't ready yet anyway).
                wm = vps_pool.tile([128, 256], f32, name="vp", tag="vp")
                for wi in range(13):
                    nc.tensor.matmul(wm[:, :], lhsT=hsT[:, 0, 0:128],
                                     rhs=hsT[:, 0, :],
                                     start=(wi == 0), stop=(wi == 12))

            # ---- attention (transpose-free, Z^T layout, pipelined pairs) --
            with tc.tile_pool(name="zps", bufs=2, space="PSUM") as z_pool, \
                 tc.tile_pool(name="dps", bufs=1, space="PSUM") as d_pool, \
                 tc.tile_pool(name="aps", bufs=2, space="PSUM") as a_pool, \
                 tc.tile_pool(name="rps", bufs=1, space="PSUM") as r_pool, \
                 tc.tile_pool(name="expz", bufs=4) as e_pool, \
                 tc.tile_pool(name="reps", bufs=2) as rp_pool, \
                 tc.tile_pool(name="fo", bufs=2) as f_pool:

                ezs = {}

                def emit_front(b):
                    # z matmuls + one fused exp over all (nh, h).  The two
                    # heads occupy disjoint 64-row strips of the PE (qt/kt
                    # partition = nh*64+a) and write different psum banks,
                    # so each (nh0, nh1) pair runs concurrently.
                    zt = z_pool.tile([128, 2, 2, 256], f32, name="zt",
                                     tag="zt")
                    for h in range(2):
                        for nh in range(NH):
                            nc.tensor.matmul(
                                zt[:, nh, h, :],
                                lhsT=kt_[nh * 64:(nh + 1) * 64, h, b, :],
                                rhs=qt[nh * 64:(nh + 1) * 64, :, b, :],
                                start=True, stop=True)
                    ez = e_pool.tile([128, 2, 2, 256], bf16, name="ez",
                                     tag="ez")
                    ezs[b] = ez
                    nc.scalar.activation(
                        ez.rearrange("p n h t -> p (n h t)"),
                        zt.rearrange("p n h t -> p (n h t)"), Exp)

                def emit_back(b):
                    ez = ezs.pop(b)
                    # denominators replicated onto the right partition
                    # halves directly by the PE; h-outer so the two nh
                    # col-strips are adjacent in the queue.
                    dpr = d_pool.tile([128, 256], f32, name="dpr", tag="dpr")
                    for h in range(2):
                        for nh in range(NH):
                            nc.tensor.matmul(
                                dpr[nh * 64:(nh + 1) * 64, :],
                                lhsT=ones_bf[:, :],
                                rhs=ez[:, nh, h, :],
                                start=(h == 0), stop=(h == 1),
                                tile_position=(0, nh * 64))
                    rep = rp_pool.tile([128, 256], f32, name="rep", tag="rep")
                    nc.vector.reciprocal_approx_fast(rep[:, :], dpr[:, :])
                    av = a_pool.tile([128, 256], f32, name="av", tag="av")
                    for kk in range(2):
                        for nh in range(NH):
                            bn = b * NH + nh
                            nc.tensor.matmul(
                                av[nh * 64:(nh + 1) * 64, :],
                                lhsT=v_all[:, bn, kk, :],
                                rhs=ez[:, nh, kk, :],
                                start=(kk == 0), stop=(kk == 1),
                                tile_position=(0, nh * 64))
                    nc.vector.tensor_mul(
                        ut[:, b, :, :].rearrange("p a c -> p (a c)"),
                        av[:, :], rep[:, :])

                rps = {}

                def emit_resid_half(bg, jh):
                    # residual per completed 4-batch group, DEFERRED one
                    # pair so the PE never waits on the DVE normalize, and
                    # SPLIT across two loop iterations (one jh accumulation
                    # step each) to balance the PE load per exp interval.
                    # Col-tiled (e-halves concurrent) so it shares the
                    # den/AV 128x64 mode -- no extra PE mode-switch drain.
                    # relu+bias on VectorE so ScalarE only runs exp.
                    if jh == 0:
                        rps[bg] = r_pool.tile([128, 512], f32, name="rp",
                                              tag="rp")
                    rp = rps[bg]
                    for eh in range(2):
                        nc.tensor.matmul(
                            rp[eh * 64:(eh + 1) * 64, :],
                            lhsT=wres_sb[:, jh, eh * 64:(eh + 1) * 64],
                            rhs=ut[:, bg * 4:(bg + 1) * 4, jh, :],
                            start=(jh == 0), stop=(jh == 1),
                            tile_position=(0, eh * 64))
                    if jh == 1:
                        del rps[bg]
                        fo = f_pool.tile([128, 512], f32, name="fo", tag="fo")
                        nc.vector.tensor_scalar(
                            fo[:, :], rp[:, :], bias_sb[:, :], 0.0, Add, Max)
                        nc.sync.dma_start(
                            out_d[:, bg * 512:(bg + 1) * 512], fo[:, :])

                for p in range(BC // 2 + 3):
                    if p < BC // 2:
                        emit_front(2 * p)
                        emit_front(2 * p + 1)
                    if p == 0:
                        # prologue filler: covers the first exp's latency
                        # (z(b2) WARs on exp(b0) via the 2-deep zt pool)
                        wm2 = d_pool.tile([128, 256], f32, name="dpr",
                                          tag="dpr")
                        for wi in range(7):
                            nc.tensor.matmul(wm2[:, :], lhsT=hsT[:, 0, 0:128],
                                             rhs=hsT[:, 0, :],
                                             start=(wi == 0), stop=(wi == 6))
                    if 1 <= p <= BC // 2:
                        emit_back(2 * (p - 1))
                        emit_back(2 * (p - 1) + 1)
                    if p >= 3 and (p - 3) % 2 == 0:
                        emit_resid_half((p - 3) // 2, 0)
                    if p >= 4 and (p - 4) % 2 == 0:
                        emit_resid_half((p - 4) // 2, 1)
    nc.compile()
    return nc


def _get_nc():
    global _NC_CACHE
    if _NC_CACHE is None:
        _NC_CACHE = build_bass()
    return _NC_CACHE


def _prep_weight(W):
    # (CD, ND) -> (128, TTILES, KTILES*128): [p, t, kt*128+j] = W[kt*128+p, t*128+j]
    return np.ascontiguousarray(
        W.astype(np.float16).reshape(KTILES, 128, TTILES, 128)
        .transpose(1, 2, 0, 3).reshape(128, TTILES, KTILES * 128))


def make_in_maps(Hs, Wq, Wk, Wv, Wres_w, Wres_b):
    wq16 = _prep_weight(Wq)
    wk16 = _prep_weight(Wk)
    # block-diagonal Wv: [128, 256], partitions (pi, e), cols (pi, 2A)
    wv16 = np.zeros((128, 256), np.float16)
    wv16[0:64, 0:128] = Wv.astype(np.float16)
    wv16[64:128, 128:256] = Wv.astype(np.float16)
    # block-diagonal Wres: [p=(nh,a), jh, col=(nh,e)]
    wres16 = np.zeros((128, 2, 128), np.float16)
    for jh in range(2):
        for nh in range(2):
            wres16[nh * 64:(nh + 1) * 64, jh, nh * 64:(nh + 1) * 64] = \
                Wres_w[jh * 64:(jh + 1) * 64, :].astype(np.float16)
    bias = np.tile(Wres_b.astype(np.float32), 2).reshape(128, 1)
    hs16 = Hs.astype(np.float16)
    maps = []
    for c in range(NCORES):
        sh = hs16[c * BC:(c + 1) * BC]                      # (BC, S, CD)
        hs2d = sh.reshape(ROWS, CD)
        hst = np.ascontiguousarray(
            hs2d.reshape(ROWS, KTILES, 128).transpose(2, 1, 0))
        # v rows in sigma' order (f*4+sp):
        # hsv[pi*64+e, q, f*4+sp] = Hs[b, nh*4+sp, f, e]; bn = 2q+pi = b*NH+nh
        arr = sh.reshape(NB, 4, F, E).transpose(0, 2, 1, 3).reshape(NB, 128, E)
        hsv = np.ascontiguousarray(
            arr.reshape(NB // 2, 2, 128, E).transpose(1, 3, 0, 2)
            .reshape(128, NB // 2, 128))
        maps.append({
            "hst": hst, "hsv": hsv,
            "wq": wq16, "wk": wk16, "wv": wv16, "wres": wres16, "bias": bias,
        })
    return maps


def _unpack_out(o):
    # o: (128, 4096) = (nh*64+e, bg, b4, f, sp) -> (BC, S, F*E)
    o = o.reshape(NH, E, BC // 4, 4, F, 4)
    return np.ascontiguousarray(
        o.transpose(2, 3, 0, 5, 4, 1)).reshape(BC, S, F * E)


def kernel(Hs, Wq, Wk, Wv, Wres_w, Wres_b):
    from concourse.bass_utils import run_bass_kernel_spmd
    nc = _get_nc()
    in_maps = make_in_maps(Hs, Wq, Wk, Wv, Wres_w, Wres_b)
    res = run_bass_kernel_spmd(nc, in_maps, list(range(NCORES)))
    out = np.concatenate(
        [_unpack_out(np.asarray(res.results[c]["out"]))
         for c in range(NCORES)], axis=0)
    return out.astype(np.float32)


if __name__ == "__main__":
    nc = build_bass()
    print("built OK; instructions:",
          sum(len(bb.instructions) for fn in nc.m.functions
              for bb in fn.blocks))
